# revision 55
# baseline (speedup 1.0000x reference)
"""Block-causal attention (B=4, N=2048, C=1024, H=16, block=128) on 8 TRN2
NeuronCores — bf16 edition.

Sharding: core = 2*b + g (b in 0..3 batches, g in 0..1 head-groups of 8
heads). Feature-major q/k, token-major v, out^T attention accumulation,
per-core partial projection; host sums the two half-feature partials.

- All matmuls bf16 (fp8 e4m3 fails the 2e-2 gate: any fp8 stage measures
  4e-2..1e-1 max-rel-err in numpy simulation — flags kept for reference).
- QK pairs use 64-row contraction at base partitions 0/64, so the two
  heads' score matmuls run concurrently in separate PE row groups.
- attn@v uses a ones-augmented v (M=65): row 64 of each AV accumulator IS
  the softmax denominator — no separate denominator pass.
- Normalization: both heads' denominator rows stage at partitions 0/32 of
  a zeroed [64, 512] tile; ONE block matmul (host [64, 128] replicator)
  replicates h0 to rows 0:64 / h1 to 64:128, one [128, 512] reciprocal,
  two muls into the bf16 `at` tile that feeds proj. rr lives in the ps_mm
  pool so this chain never blocks the next unit's QK score matmuls.
- exp runs on ACT (exp(S)/32 with per-partition bias); every 4th tile is
  offloaded to DVE via a Schraudolph bitcast exp (i16 saturating convert).
- x chunks and wk load via 4 sub-DMAs each (fine-grained sems for early
  start); wq/wv/wp load in one dma_start each.
- Tail: last chunk's proj runs through the idle ps_s pool with [128, 1024]
  merged evacuations and stores.
"""

import numpy as np
import ml_dtypes
from collections import deque
from contextlib import ExitStack

B, N, C, H, HD = 4, 2048, 1024, 16, 64
HPC = 8               # heads per core
F = HPC * HD          # 512 features per core
NCORES = 8
SCALE = float(HD) ** -0.5
NT = N // 128         # 16 token tiles
NCH = 4               # token chunks of 512

QKV_FP8 = False
ET_FP8 = False
PROJ_FP8 = False
WSCALE = 32.0         # host pre-scale on wq/wk/wv/wp before fp8 quant
DVE_EXP_MOD = 4       # every DVE_EXP_MOD-th exp tile runs on DVE (0 = off)

LNK = 3.4657359027997265  # ln(32): exp(s)/32 keeps max logit 8.06 under fp8e4 max 240
LOG2E = 1.4426950408889634

_CACHE = {}


def _build():
    import concourse.mybir as mybir
    import concourse.tile as tile
    from concourse import bacc

    f32 = mybir.dt.float32
    f32r = mybir.dt.float32r
    bf16 = mybir.dt.bfloat16
    f8 = mybir.dt.float8e4
    u8 = mybir.dt.uint8
    i16 = mybir.dt.int16
    Exp = mybir.ActivationFunctionType.Exp
    DR = mybir.MatmulPerfMode.DoubleRow
    MULT = mybir.AluOpType.mult
    ADD = mybir.AluOpType.add

    qk_ws = WSCALE * WSCALE if QKV_FP8 else 1.0
    scale_eff = SCALE / qk_ws
    et_dt = f8 if ET_FP8 else bf16
    at_dt = f8 if PROJ_FP8 else bf16
    K8 = scale_eff * LOG2E * 8.0
    B8 = (7.0 - 5.0) * 8.0 - 0.5
    K16 = scale_eff * LOG2E * 128.0
    B16 = 127.0 * 128.0 - 0.0579 * 128.0 - 5.0 * 128.0  # incl. exp/32

    nc = bacc.Bacc("TRN2", target_bir_lowering=False, debug=False,
                   num_devices=NCORES)

    x_dt = f8 if QKV_FP8 else bf16
    xT = nc.dram_tensor("xT", [C, N], x_dt, kind="ExternalInput")
    wq = nc.dram_tensor("wq", [C, F], x_dt, kind="ExternalInput")
    wk = nc.dram_tensor("wk", [C, F], x_dt, kind="ExternalInput")
    wv = nc.dram_tensor("wv", [C, F], x_dt, kind="ExternalInput")
    wp = nc.dram_tensor("wp", [F, C], at_dt, kind="ExternalInput")
    ones2d = nc.dram_tensor("ones2", [64, 128], bf16, kind="ExternalInput")
    out = nc.dram_tensor("out", [N, C], f32, kind="ExternalOutput")

    with tile.TileContext(nc) as tc, ExitStack() as ctx:
        persist = ctx.enter_context(tc.tile_pool(name="persist", bufs=1))
        xt_pool = ctx.enter_context(tc.tile_pool(name="xt", bufs=2))
        qt_pool = ctx.enter_context(tc.tile_pool(name="qt", bufs=2))
        et_pool = ctx.enter_context(tc.tile_pool(name="et", bufs=4))
        sm_pool = ctx.enter_context(tc.tile_pool(name="sums", bufs=2))
        rcf_pool = ctx.enter_context(tc.tile_pool(name="rcf", bufs=2))
        ost_pool = ctx.enter_context(tc.tile_pool(name="ost", bufs=2))
        ps_mm = ctx.enter_context(tc.tile_pool(name="ps_mm", bufs=2, space="PSUM"))
        ps_s = ctx.enter_context(tc.tile_pool(name="ps_s", bufs=2, space="PSUM"))
        ps_av = ctx.enter_context(tc.tile_pool(name="ps_av", bufs=1, space="PSUM"))

        # ---- persistent weights ----
        if QKV_FP8:
            wq_t = [persist.tile([128, 2, F], f8, name=f"wq{p}", tag=f"wq{p}")
                    for p in range(4)]
            wk_t = [persist.tile([128, 2, F], f8, name=f"wk{p}", tag=f"wk{p}")
                    for p in range(4)]
            wv_t = [persist.tile([128, 2, F], f8, name=f"wv{p}", tag=f"wv{p}")
                    for p in range(4)]
        else:
            # fused [128, 8, F] so each weight tensor loads in ONE dma_start
            wq_t = persist.tile([128, 8, F], bf16, name="wqf", tag="wqf")
            wk_t = persist.tile([128, 8, F], bf16, name="wkf", tag="wkf")
            wv_t = persist.tile([128, 8, F], bf16, name="wvf", tag="wvf")
        if PROJ_FP8:
            wp_t = [persist.tile([128, 2, C], f8, name=f"wp{p}", tag=f"wp{p}")
                    for p in range(2)]
        else:
            wp_t = persist.tile([128, 4, C], bf16, name="wpf", tag="wpf")
        # block replicator: row 0 is 1 on cols 0:64, row 32 is 1 on cols
        # 64:128, 0 elsewhere — one K=64 matmul replicates both heads'
        # denominators at once. The head sums stage at partitions 0 and 32
        # (engine writes must start at 32-aligned partitions); the other
        # rows stay 0 from the startup memset.
        ones2_t = persist.tile([64, 128], bf16, name="ones2", tag="ones2")
        # garbage operand for the HAM warm-up burst (memset once, never
        # DMA'd, so the warm-up matmuls depend on nothing but the memset)
        warm_t = persist.tile([128, 512], bf16, name="warm", tag="warm")
        sums_t = [persist.tile([64, 512], bf16, name=f"sums{i}",
                               tag=f"sums{i}") for i in range(2)]
        bias_t = persist.tile([128, 1], f32, name="bias", tag="bias")

        # persistent k^T (bf16) and ones-augmented token-major v (pairs)
        kt_t = [[persist.tile([128, 512], bf16, name=f"kT{hp}_{jc}",
                              tag=f"kT{hp}_{jc}")
                 for jc in range(NCH)] for hp in range(4)]
        # 66-wide per-head v slices: 64 feature cols + ones col + pad col
        # (even head offsets/strides; dual-fp8 ldweights requires them)
        v_t = [persist.tile([128, 2, 8 * 66], et_dt, name=f"v{jp}",
                            tag=f"v{jp}") for jp in range(NT // 2)]
        at_t = [persist.tile([128, 4, 512], at_dt, name=f"at{c}", tag=f"at{c}")
                for c in range(NCH)]

        def load_weights():
            nc.gpsimd.dma_start(ones2_t[:], ones2d[:, :])
            nc.vector.memset(sums_t[0][:], 0.0)
            nc.vector.memset(sums_t[1][:], 0.0)
            nc.vector.memset(bias_t[:], -LNK)
            for jp in range(NT // 2):
                # col 64 of each head's 65-wide v slice must be 1.0; the v
                # copies later only overwrite cols 0:64, so setting cols
                # 64:66 (ones + pad) up-front leaves them in place forever.
                # Vector queue: keeps gpsimd free to start weight DMAs.
                ones_cols = v_t[jp][:].rearrange(
                    "p i (h e) -> p i h e", e=66)[:, :, :, 64:66]
                nc.vector.memset(ones_cols, 1.0)
            # wk gates the very first matmuls: split 4-way so the k=0 pair
            # lands early. wv/wq/wp are needed later; one dma_start each.
            wkr = wk[:, :].rearrange("(k p) f -> p k f", p=128)
            wvr = wv[:, :].rearrange("(k p) f -> p k f", p=128)
            wqr = wq[:, :].rearrange("(k p) f -> p k f", p=128)
            for q in range(4):
                nc.scalar.dma_start(wk_t[:, 2 * q:2 * q + 2, :],
                                    wkr[:, 2 * q:2 * q + 2, :])
            for q in range(4):
                nc.gpsimd.dma_start(wv_t[:, 2 * q:2 * q + 2, :],
                                    wvr[:, 2 * q:2 * q + 2, :])
            for q in range(2):
                nc.sync.dma_start(wq_t[:, 4 * q:4 * q + 4, :],
                                  wqr[:, 4 * q:4 * q + 4, :])
            # wp is needed only at the first proj fillers; queue it on the
            # scalar ring BEHIND wk so it doesn't steal early DMA bandwidth
            # from the xt/wk/wv transfers that gate the first matmuls.
            nc.scalar.dma_start(
                wp_t[:], wp[:, :].rearrange("(k p) c -> p k c", p=128))

        qt_state = {c: [] for c in range(NCH)}
        exp_ctr = [0]
        mm_ctr = [0]
        unit_ctr = [0]

        def mm_tile(c):
            # chunk 0 runs before any attention, so the ss banks are idle;
            # alternating into them deepens the qkv pipeline 2 -> 4 buffers.
            mm_ctr[0] += 1
            if c == 0 and mm_ctr[0] % 2 == 0:
                return ps_s.tile([128, 1024], f32, name="s", tag="s")[:, 0:512]
            return ps_mm.tile([128, 512], f32, name="mm", tag="mm")[:]

        def qkv_units(c):
            c0 = c * 512
            xt_c = []

            def load():
                # 4 dma_starts per chunk: fine-grained sems let the k=0
                # matmuls start while later k-slices are still in flight.
                xt = xt_pool.tile([128, 8, 512], bf16, name="xt", tag="xt")
                xr = xT[:, c0:c0 + 512].rearrange("(k p) n -> p k n", p=128)
                for q in range(4):
                    nc.sync.dma_start(xt[:, 2 * q:2 * q + 2, :],
                                      xr[:, 2 * q:2 * q + 2, :])
                xt_c.append(xt)

            def qk_mm(ps, w_t, hp):
                for k in range(8):
                    nc.tensor.matmul(ps[:],
                                     w_t[:, k, hp * 128:(hp + 1) * 128],
                                     xt_c[0][:, k, :],
                                     start=(k == 0), stop=(k == 7))

            def q_group(hp):
                def emit():
                    ps = mm_tile(c)
                    qk_mm(ps, wq_t, hp)
                    qt = qt_pool.tile([128, 512], bf16, name=f"qT{hp}",
                                      tag=f"qT{hp}")
                    nc.vector.tensor_copy(qt[:], ps[:])
                    qt_state[c].append(qt)
                return emit

            def k_group(hp):
                def emit():
                    ps = mm_tile(c)
                    qk_mm(ps, wk_t, hp)
                    nc.vector.tensor_copy(kt_t[hp][c][:], ps[:])
                return emit

            def v_group(tl):
                def emit():
                    t = 4 * c + tl
                    ps = mm_tile(c)
                    for k in range(8):
                        nc.tensor.matmul(ps[:],
                                         xt_c[0][:, k, tl * 128:(tl + 1) * 128],
                                         wv_t[:, k, :],
                                         start=(k == 0), stop=(k == 7))
                    src = ps[:].rearrange("p (h e) -> p h e", e=64)
                    dst = v_t[t // 2][:, t % 2, :].rearrange(
                        "p (h e) -> p h e", e=66)[:, :, 0:64]
                    nc.vector.tensor_copy(dst, src)
                return emit

            units = [load]
            for hp in range(4):
                units.append(k_group(hp))
            for tl in range(4):
                units.append(v_group(tl))
            for hp in range(4):
                units.append(q_group(hp))
            return units

        def proj_units(c):
            units = []
            for tl in range(4):
                for n2 in range(2):
                    def emit(tl=tl, n2=n2):
                        t = 4 * c + tl
                        ps = ps_mm.tile([128, 512], f32, name="mm", tag="mm")
                        for k in range(4):
                            nc.tensor.matmul(
                                ps[:],
                                at_t[c][:, k, tl * 128:(tl + 1) * 128],
                                wp_t[:, k, n2 * 512:(n2 + 1) * 512],
                                start=(k == 0), stop=(k == 3))
                        ost = ost_pool.tile([128, 512], f32, name="ost",
                                            tag="ost")
                        nc.vector.tensor_copy(ost[:], ps[:])
                        nc.sync.dma_start(
                            out[t * 128:(t + 1) * 128,
                                n2 * 512:(n2 + 1) * 512],
                            ost[:])
                    units.append(emit)
            return units

        units0 = qkv_units(0)
        units0[0]()
        # HAM warm-up: ~10us of garbage matmuls (8 cold + 32 warm) bridge
        # the whole first-chunk DMA wait, so the clock gate is at 8/8 and
        # stays there when the real stream starts at ~12us. All of this
        # hides inside the unavoidable DMA window.
        nc.vector.memset(warm_t[:], 0.125)
        wps = ps_mm.tile([128, 512], f32, name="mm", tag="mm")
        for _ in range(40):
            nc.tensor.matmul(wps[:], warm_t[0:128, 0:128], warm_t[:, 0:512],
                             start=True, stop=True)
        load_weights()
        for u in units0[1:]:
            u()

        def emit_exp(ss, et, i, vco):
            src = ss[:].rearrange("p (h q) -> p h q", h=2)[:, :, vco:512]
            dst = et[:, i, :].rearrange("p (h q) -> p h q", h=2)[:, :, vco:512]
            exp_ctr[0] += 1
            on_dve = DVE_EXP_MOD and (exp_ctr[0] % DVE_EXP_MOD == 0)
            if on_dve:
                if ET_FP8:
                    nc.vector.tensor_scalar(dst.bitcast(u8), src, K8, B8,
                                            MULT, ADD)
                else:
                    nc.vector.tensor_scalar(dst.bitcast(i16), src, K16, B16,
                                            MULT, ADD)
            else:
                nc.scalar.activation(dst, src, Exp, bias=bias_t[:],
                                     scale=scale_eff)

        def attn_unit(c, hp, fillers, stride=2):
            njp = 2 * c + 2
            qt_c = qt_state[c]
            av = [ps_av.tile([128, 512], f32, name=f"av{h}", tag=f"av{h}")
                  for h in range(2)]
            for jp in range(njp):
                first, last = (jp == 0), (jp == njp - 1)
                et = et_pool.tile([128, 2, 1024], et_dt, name="e", tag="e")
                vcos = []
                for i in range(2):
                    j = 2 * jp + i
                    jd = j - 4 * c
                    vco = jd * 128 if jd > 0 else 0
                    vcos.append(vco)
                    kt = kt_t[hp][j // 4]
                    kc = (j % 4) * 128
                    ss = ps_s.tile([128, 1024], f32, name="s", tag="s")
                    nc.tensor.matmul(ss[:, vco:512],
                                     kt[0:64, kc:kc + 128],
                                     qt_c[hp][0:64, vco:512],
                                     start=True, stop=True)
                    nc.tensor.matmul(ss[:, 512 + vco:1024],
                                     kt[64:128, kc:kc + 128],
                                     qt_c[hp][64:128, vco:512],
                                     start=True, stop=True)
                    emit_exp(ss, et, i, vco)
                # i-major: both heads' AV for key tile i=0 are queued before
                # the exp(i=1)-dependent matmuls, so the PE FIFO never holds
                # a blocked matmul ahead of an unblocked one.
                for i in range(2):
                    vco = vcos[i]
                    for h in range(2):
                        gh = 2 * hp + h
                        co = h * 512
                        vsl = v_t[jp][:, i, gh * 66:gh * 66 + 65]
                        esl = et[:, i, co + vco:co + 512]
                        nc.tensor.matmul(av[h][0:65, vco:512], vsl, esl,
                                         start=(first and i == 0),
                                         stop=(last and i == 1))
                if jp % stride == stride - 1 and fillers:
                    fillers.popleft()()
            # keep the PE fed while the DVE sums-copy / recip chain runs:
            # pop fillers BEFORE the rc_rep matmuls (PE executes in emission
            # order, so anything emitted after rc_rep would stall behind it)
            if fillers:
                fillers.popleft()()
            # ---- normalization: row 64 of av[h] is the denominator ----
            # both heads' sums stage at partitions 0/32 of one tile; one
            # block matmul replicates h0 to rows 0:64 and h1 to rows
            # 64:128, so the reciprocal is a single [128, 512] pass.
            unit_ctr[0] += 1
            sums = sums_t[unit_ctr[0] % 2]
            # h0 copy on DVE, h1 copy + reciprocal on ACT: the chain hops
            # engines so each step starts on its semaphore instead of
            # queueing, and the av banks free up sooner for the next unit.
            nc.vector.tensor_copy(sums[0:1, :], av[0][64:65, :])
            nc.scalar.activation(sums[32:33, :], av[1][64:65, :],
                                 mybir.ActivationFunctionType.Copy)
            rr = ps_mm.tile([128, 512], f32, name="rr", tag="mm")
            nc.tensor.matmul(rr[:], ones2_t[:], sums[:],
                             start=True, stop=True)
            rcf = rcf_pool.tile([128, 512], f32, name="rcf", tag="rcf")
            nc.vector.reciprocal_approx_fast(rcf[:], rr[:])
            for h in range(2):
                nc.vector.tensor_mul(at_t[c][h * 64:(h + 1) * 64, hp, :],
                                     av[h][0:64, :],
                                     rcf[h * 64:(h + 1) * 64, :])

        phases = [
            ([(0, 0), (0, 1), (0, 2), (0, 3)], qkv_units(1)),
            ([(1, 0), (1, 1), (1, 2), (1, 3)], qkv_units(2)),
            ([(2, 0), (2, 1), (2, 2), (2, 3)], qkv_units(3)),
            ([(3, 0), (3, 1), (3, 2), (3, 3)],
             proj_units(0) + proj_units(1) + proj_units(2)),
        ]
        for units, filler_list in phases:
            fillers = deque(filler_list)
            total_jp = sum(2 * c + 2 for c, hp in units)
            stride = max(1, -(-total_jp // max(1, len(filler_list))))
            for (c, hp) in units:
                attn_unit(c, hp, fillers, stride)
            while fillers:
                fillers.popleft()()

        # tail: last chunk's proj through the now-idle ps_s pool. Each
        # 512-wide half evacuates and stores as soon as its 4-matmul chain
        # stops, so the final evac/store drains only half a tile.
        c = NCH - 1
        for tl in range(4):
            t = 4 * c + tl
            ps = ps_s.tile([128, 1024], f32, name="s", tag="s")
            ost = ost_pool.tile([128, 1024], f32, name="ost2", tag="ost2")
            # k0..k2 of BOTH n2 chains first: at[:, 3, :] comes from the
            # last attention unit's normalize, so deferring the k=3
            # matmuls hides that wait behind six ready matmuls.
            for k in range(3):
                for n2 in range(2):
                    nc.tensor.matmul(
                        ps[:, n2 * 512:(n2 + 1) * 512],
                        at_t[c][:, k, tl * 128:(tl + 1) * 128],
                        wp_t[:, k, n2 * 512:(n2 + 1) * 512],
                        start=(k == 0), stop=False)
            for n2 in range(2):
                nc.tensor.matmul(
                    ps[:, n2 * 512:(n2 + 1) * 512],
                    at_t[c][:, 3, tl * 128:(tl + 1) * 128],
                    wp_t[:, 3, n2 * 512:(n2 + 1) * 512],
                    start=False, stop=True)
                nc.vector.tensor_copy(ost[:, n2 * 512:(n2 + 1) * 512],
                                      ps[:, n2 * 512:(n2 + 1) * 512])
                q = nc.sync if n2 == 0 else nc.scalar
                q.dma_start(out[t * 128:(t + 1) * 128,
                                n2 * 512:(n2 + 1) * 512],
                            ost[:, n2 * 512:(n2 + 1) * 512])

    nc.compile()
    return nc


def _get_nc():
    if "nc" not in _CACHE:
        _CACHE["nc"] = _build()
    return _CACHE["nc"]


def _ones2_host():
    o = np.zeros((64, 128), dtype=ml_dtypes.bfloat16)
    o[0, 0:64] = 1.0
    o[32, 64:128] = 1.0
    return o


def _in_maps(x, w_qkv, w_proj):
    f8np = ml_dtypes.float8_e4m3
    bf16np = ml_dtypes.bfloat16
    x_np = f8np if QKV_FP8 else bf16np
    at_np = f8np if PROJ_FP8 else bf16np
    wsc = WSCALE if QKV_FP8 else 1.0
    wsp = WSCALE if PROJ_FP8 else 1.0

    wr = w_qkv.reshape(C, 3, H, HD)
    wpr = w_proj.reshape(H, HD, C)
    maps = []
    for core in range(NCORES):
        b, g = core // 2, core % 2
        hs = slice(g * HPC, (g + 1) * HPC)
        maps.append({
            "xT": np.ascontiguousarray(x[b].T).astype(x_np),
            "wq": (wr[:, 0, hs, :].reshape(C, F) * wsc).astype(x_np),
            "wk": (wr[:, 1, hs, :].reshape(C, F) * wsc).astype(x_np),
            "wv": (wr[:, 2, hs, :].reshape(C, F) * wsc).astype(x_np),
            "wp": (wpr[hs].reshape(F, C) * wsp).astype(at_np),
            "ones2": _ones2_host(),
        })
    return maps


def kernel(x, w_qkv, w_proj, b_proj, _trace=False):
    from concourse.bass_utils import run_bass_kernel_spmd

    x = np.asarray(x, dtype=np.float32)
    w_qkv = np.asarray(w_qkv, dtype=np.float32)
    w_proj = np.asarray(w_proj, dtype=np.float32)
    b_proj = np.asarray(b_proj, dtype=np.float32)

    nc = _get_nc()
    in_maps = _in_maps(x, w_qkv, w_proj)
    try:
        res = run_bass_kernel_spmd(nc, in_maps, list(range(NCORES)),
                                   trace=_trace)
    except Exception:
        try:
            import ctypes
            import jax
            lib = ctypes.CDLL("/opt/axon/libaxon_pjrt.so")
            jax.devices()
            lib.axon_reset.restype = ctypes.c_int64
            lib.axon_reset()
        except Exception:
            pass
        res = run_bass_kernel_spmd(nc, in_maps, list(range(NCORES)),
                                   trace=_trace)
    unscale = 1.0
    if QKV_FP8:
        unscale *= WSCALE        # v path carries one WSCALE into at
    if PROJ_FP8:
        unscale *= WSCALE        # wp carries another
    out = np.empty((B, N, C), dtype=np.float32)
    for b in range(B):
        out[b] = res.results[2 * b]["out"] + res.results[2 * b + 1]["out"]
    if unscale != 1.0:
        out *= 1.0 / unscale
    out += b_proj.reshape(1, 1, C)
    if _trace:
        return out, res
    return out



# revision 57
# speedup vs baseline: 1.0064x; 1.0064x over previous
"""Block-causal attention (B=4, N=2048, C=1024, H=16, block=128) on 8 TRN2
NeuronCores — bf16 edition.

Sharding: core = 2*b + g (b in 0..3 batches, g in 0..1 head-groups of 8
heads). Feature-major q/k, token-major v, out^T attention accumulation,
per-core partial projection; host sums the two half-feature partials.

- All matmuls bf16 (fp8 e4m3 fails the 2e-2 gate: any fp8 stage measures
  4e-2..1e-1 max-rel-err in numpy simulation — flags kept for reference).
- QK pairs use 64-row contraction at base partitions 0/64, so the two
  heads' score matmuls run concurrently in separate PE row groups.
- attn@v uses a ones-augmented v (M=65): row 64 of each AV accumulator IS
  the softmax denominator — no separate denominator pass.
- Normalization: both heads' denominator rows stage at partitions 0/32 of
  a zeroed [64, 512] tile; ONE block matmul (host [64, 128] replicator)
  replicates h0 to rows 0:64 / h1 to 64:128, one [128, 512] reciprocal,
  two muls into the bf16 `at` tile that feeds proj. rr lives in the ps_mm
  pool so this chain never blocks the next unit's QK score matmuls.
- exp runs on ACT (exp(S)/32 with per-partition bias); every 4th tile is
  offloaded to DVE via a Schraudolph bitcast exp (i16 saturating convert).
- x chunks and wk load via 4 sub-DMAs each (fine-grained sems for early
  start); wq/wv/wp load in one dma_start each.
- Tail: last chunk's proj runs through the idle ps_s pool with [128, 1024]
  merged evacuations and stores.
"""

import numpy as np
import ml_dtypes
from collections import deque
from contextlib import ExitStack

B, N, C, H, HD = 4, 2048, 1024, 16, 64
HPC = 8               # heads per core
F = HPC * HD          # 512 features per core
NCORES = 8
SCALE = float(HD) ** -0.5
NT = N // 128         # 16 token tiles
NCH = 4               # token chunks of 512

QKV_FP8 = False
ET_FP8 = False
PROJ_FP8 = False
WSCALE = 32.0         # host pre-scale on wq/wk/wv/wp before fp8 quant
DVE_EXP_MOD = 4       # every DVE_EXP_MOD-th exp tile runs on DVE (0 = off)

LNK = 3.4657359027997265  # ln(32): exp(s)/32 keeps max logit 8.06 under fp8e4 max 240
LOG2E = 1.4426950408889634

_CACHE = {}


def _build():
    import concourse.mybir as mybir
    import concourse.tile as tile
    from concourse import bacc

    f32 = mybir.dt.float32
    f32r = mybir.dt.float32r
    bf16 = mybir.dt.bfloat16
    f8 = mybir.dt.float8e4
    u8 = mybir.dt.uint8
    i16 = mybir.dt.int16
    Exp = mybir.ActivationFunctionType.Exp
    DR = mybir.MatmulPerfMode.DoubleRow
    MULT = mybir.AluOpType.mult
    ADD = mybir.AluOpType.add

    qk_ws = WSCALE * WSCALE if QKV_FP8 else 1.0
    scale_eff = SCALE / qk_ws
    et_dt = f8 if ET_FP8 else bf16
    at_dt = f8 if PROJ_FP8 else bf16
    K8 = scale_eff * LOG2E * 8.0
    B8 = (7.0 - 5.0) * 8.0 - 0.5
    K16 = scale_eff * LOG2E * 128.0
    B16 = 127.0 * 128.0 - 0.0579 * 128.0 - 5.0 * 128.0  # incl. exp/32

    nc = bacc.Bacc("TRN2", target_bir_lowering=False, debug=False,
                   num_devices=NCORES)

    x_dt = f8 if QKV_FP8 else bf16
    xT = nc.dram_tensor("xT", [C, N], x_dt, kind="ExternalInput")
    wq = nc.dram_tensor("wq", [C, F], x_dt, kind="ExternalInput")
    wk = nc.dram_tensor("wk", [C, F], x_dt, kind="ExternalInput")
    wv = nc.dram_tensor("wv", [C, F], x_dt, kind="ExternalInput")
    wp = nc.dram_tensor("wp", [F, C], at_dt, kind="ExternalInput")
    ones2d = nc.dram_tensor("ones2", [64, 128], bf16, kind="ExternalInput")
    out = nc.dram_tensor("out", [N, C], f32, kind="ExternalOutput")

    with tile.TileContext(nc) as tc, ExitStack() as ctx:
        persist = ctx.enter_context(tc.tile_pool(name="persist", bufs=1))
        xt_pool = ctx.enter_context(tc.tile_pool(name="xt", bufs=2))
        qt_pool = ctx.enter_context(tc.tile_pool(name="qt", bufs=2))
        et_pool = ctx.enter_context(tc.tile_pool(name="et", bufs=3))
        sm_pool = ctx.enter_context(tc.tile_pool(name="sums", bufs=2))
        rcf_pool = ctx.enter_context(tc.tile_pool(name="rcf", bufs=2))
        ost_pool = ctx.enter_context(tc.tile_pool(name="ost", bufs=2))
        ps_mm = ctx.enter_context(tc.tile_pool(name="ps_mm", bufs=2, space="PSUM"))
        ps_s = ctx.enter_context(tc.tile_pool(name="ps_s", bufs=2, space="PSUM"))
        ps_av = ctx.enter_context(tc.tile_pool(name="ps_av", bufs=1, space="PSUM"))

        # ---- persistent weights ----
        if QKV_FP8:
            wq_t = [persist.tile([128, 2, F], f8, name=f"wq{p}", tag=f"wq{p}")
                    for p in range(4)]
            wk_t = [persist.tile([128, 2, F], f8, name=f"wk{p}", tag=f"wk{p}")
                    for p in range(4)]
            wv_t = [persist.tile([128, 2, F], f8, name=f"wv{p}", tag=f"wv{p}")
                    for p in range(4)]
        else:
            # fused [128, 8, F] so each weight tensor loads in ONE dma_start
            wq_t = persist.tile([128, 8, F], bf16, name="wqf", tag="wqf")
            wk_t = persist.tile([128, 8, F], bf16, name="wkf", tag="wkf")
            wv_t = persist.tile([128, 8, F], bf16, name="wvf", tag="wvf")
        if PROJ_FP8:
            wp_t = [persist.tile([128, 2, C], f8, name=f"wp{p}", tag=f"wp{p}")
                    for p in range(2)]
        else:
            wp_t = persist.tile([128, 4, C], bf16, name="wpf", tag="wpf")
        # block replicator: row 0 is 1 on cols 0:64, row 32 is 1 on cols
        # 64:128, 0 elsewhere — one K=64 matmul replicates both heads'
        # denominators at once. The head sums stage at partitions 0 and 32
        # (engine writes must start at 32-aligned partitions); the other
        # rows stay 0 from the startup memset.
        ones2_t = persist.tile([64, 128], bf16, name="ones2", tag="ones2")
        # garbage operand for the HAM warm-up burst (memset once, never
        # DMA'd, so the warm-up matmuls depend on nothing but the memset)
        warm_t = persist.tile([128, 512], bf16, name="warm", tag="warm")
        sums_t = [persist.tile([64, 512], bf16, name=f"sums{i}",
                               tag=f"sums{i}") for i in range(2)]
        bias_t = persist.tile([128, 1], f32, name="bias", tag="bias")

        # persistent k^T (bf16) and ones-augmented token-major v (pairs)
        kt_t = [[persist.tile([128, 512], bf16, name=f"kT{hp}_{jc}",
                              tag=f"kT{hp}_{jc}")
                 for jc in range(NCH)] for hp in range(4)]
        # 66-wide per-head v slices: 64 feature cols + ones col + pad col
        # (even head offsets/strides; dual-fp8 ldweights requires them)
        v_t = [persist.tile([128, 2, 8 * 66], et_dt, name=f"v{jp}",
                            tag=f"v{jp}") for jp in range(NT // 2)]
        at_t = [persist.tile([128, 4, 512], at_dt, name=f"at{c}", tag=f"at{c}")
                for c in range(NCH)]

        def load_weights():
            nc.gpsimd.dma_start(ones2_t[:], ones2d[:, :])
            nc.vector.memset(sums_t[0][:], 0.0)
            nc.vector.memset(sums_t[1][:], 0.0)
            nc.vector.memset(bias_t[:], -LNK)
            for jp in range(NT // 2):
                # col 64 of each head's 65-wide v slice must be 1.0; the v
                # copies later only overwrite cols 0:64, so setting cols
                # 64:66 (ones + pad) up-front leaves them in place forever.
                # Vector queue: keeps gpsimd free to start weight DMAs.
                ones_cols = v_t[jp][:].rearrange(
                    "p i (h e) -> p i h e", e=66)[:, :, :, 64:66]
                nc.vector.memset(ones_cols, 1.0)
            # wk gates the very first matmuls: split 4-way so the k=0 pair
            # lands early. wv/wq/wp are needed later; one dma_start each.
            wkr = wk[:, :].rearrange("(k p) f -> p k f", p=128)
            wvr = wv[:, :].rearrange("(k p) f -> p k f", p=128)
            wqr = wq[:, :].rearrange("(k p) f -> p k f", p=128)
            for q in range(4):
                nc.scalar.dma_start(wk_t[:, 2 * q:2 * q + 2, :],
                                    wkr[:, 2 * q:2 * q + 2, :])
            for q in range(4):
                nc.gpsimd.dma_start(wv_t[:, 2 * q:2 * q + 2, :],
                                    wvr[:, 2 * q:2 * q + 2, :])
            for q in range(2):
                nc.sync.dma_start(wq_t[:, 4 * q:4 * q + 4, :],
                                  wqr[:, 4 * q:4 * q + 4, :])
            # wp is needed only at the first proj fillers; queue it on the
            # scalar ring BEHIND wk so it doesn't steal early DMA bandwidth
            # from the xt/wk/wv transfers that gate the first matmuls.
            nc.scalar.dma_start(
                wp_t[:], wp[:, :].rearrange("(k p) c -> p k c", p=128))

        qt_state = {c: [] for c in range(NCH)}
        exp_ctr = [0]
        mm_ctr = [0]
        unit_ctr = [0]

        def mm_tile(c):
            # chunk 0 runs before any attention, so the ss banks are idle;
            # alternating into them deepens the qkv pipeline 2 -> 4 buffers.
            mm_ctr[0] += 1
            if c == 0 and mm_ctr[0] % 2 == 0:
                return ps_s.tile([128, 1024], f32, name="s", tag="s")[:, 0:512]
            return ps_mm.tile([128, 512], f32, name="mm", tag="mm")[:]

        def qkv_units(c):
            c0 = c * 512
            xt_c = []

            def load():
                # 4 dma_starts per chunk: fine-grained sems let the k=0
                # matmuls start while later k-slices are still in flight.
                xt = xt_pool.tile([128, 8, 512], bf16, name="xt", tag="xt")
                xr = xT[:, c0:c0 + 512].rearrange("(k p) n -> p k n", p=128)
                for q in range(4):
                    nc.sync.dma_start(xt[:, 2 * q:2 * q + 2, :],
                                      xr[:, 2 * q:2 * q + 2, :])
                xt_c.append(xt)

            def qk_mm(ps, w_t, hp):
                for k in range(8):
                    nc.tensor.matmul(ps[:],
                                     w_t[:, k, hp * 128:(hp + 1) * 128],
                                     xt_c[0][:, k, :],
                                     start=(k == 0), stop=(k == 7))

            def q_group(hp):
                def emit():
                    ps = mm_tile(c)
                    qk_mm(ps, wq_t, hp)
                    qt = qt_pool.tile([128, 512], bf16, name=f"qT{hp}",
                                      tag=f"qT{hp}")
                    nc.vector.tensor_copy(qt[:], ps[:])
                    qt_state[c].append(qt)
                return emit

            def k_group(hp):
                def emit():
                    ps = mm_tile(c)
                    qk_mm(ps, wk_t, hp)
                    nc.vector.tensor_copy(kt_t[hp][c][:], ps[:])
                return emit

            def v_group(tl):
                def emit():
                    t = 4 * c + tl
                    ps = mm_tile(c)
                    for k in range(8):
                        nc.tensor.matmul(ps[:],
                                         xt_c[0][:, k, tl * 128:(tl + 1) * 128],
                                         wv_t[:, k, :],
                                         start=(k == 0), stop=(k == 7))
                    src = ps[:].rearrange("p (h e) -> p h e", e=64)
                    dst = v_t[t // 2][:, t % 2, :].rearrange(
                        "p (h e) -> p h e", e=66)[:, :, 0:64]
                    nc.vector.tensor_copy(dst, src)
                return emit

            units = [load]
            for hp in range(4):
                units.append(k_group(hp))
            for tl in range(4):
                units.append(v_group(tl))
            for hp in range(4):
                units.append(q_group(hp))
            return units

        def proj_units(c):
            units = []
            for tl in range(4):
                for n2 in range(2):
                    def emit(tl=tl, n2=n2):
                        t = 4 * c + tl
                        ps = ps_mm.tile([128, 512], f32, name="mm", tag="mm")
                        for k in range(4):
                            nc.tensor.matmul(
                                ps[:],
                                at_t[c][:, k, tl * 128:(tl + 1) * 128],
                                wp_t[:, k, n2 * 512:(n2 + 1) * 512],
                                start=(k == 0), stop=(k == 3))
                        ost = ost_pool.tile([128, 512], f32, name="ost",
                                            tag="ost")
                        nc.vector.tensor_copy(ost[:], ps[:])
                        nc.sync.dma_start(
                            out[t * 128:(t + 1) * 128,
                                n2 * 512:(n2 + 1) * 512],
                            ost[:])
                    units.append(emit)
            return units

        units0 = qkv_units(0)
        units0[0]()
        # HAM warm-up: ~10us of garbage matmuls (8 cold + 32 warm) bridge
        # the whole first-chunk DMA wait, so the clock gate is at 8/8 and
        # stays there when the real stream starts at ~12us. All of this
        # hides inside the unavoidable DMA window.
        nc.vector.memset(warm_t[:], 0.125)
        wps = ps_mm.tile([128, 512], f32, name="mm", tag="mm")
        for _ in range(40):
            nc.tensor.matmul(wps[:], warm_t[0:128, 0:128], warm_t[:, 0:512],
                             start=True, stop=True)
        load_weights()
        for u in units0[1:]:
            u()

        def emit_exp(ss, et, i, vco):
            src = ss[:].rearrange("p (h q) -> p h q", h=2)[:, :, vco:512]
            dst = et[:, i, :].rearrange("p (h q) -> p h q", h=2)[:, :, vco:512]
            exp_ctr[0] += 1
            on_dve = DVE_EXP_MOD and (exp_ctr[0] % DVE_EXP_MOD == 0)
            if on_dve:
                if ET_FP8:
                    nc.vector.tensor_scalar(dst.bitcast(u8), src, K8, B8,
                                            MULT, ADD)
                else:
                    nc.vector.tensor_scalar(dst.bitcast(i16), src, K16, B16,
                                            MULT, ADD)
            else:
                nc.scalar.activation(dst, src, Exp, bias=bias_t[:],
                                     scale=scale_eff)

        def attn_unit(c, hp, fillers, stride=2):
            njp = 2 * c + 2
            qt_c = qt_state[c]
            av = [ps_av.tile([128, 512], f32, name=f"av{h}", tag=f"av{h}")
                  for h in range(2)]
            for jp in range(njp):
                first, last = (jp == 0), (jp == njp - 1)
                et = et_pool.tile([128, 2, 1024], et_dt, name="e", tag="e")
                vcos = []
                for i in range(2):
                    j = 2 * jp + i
                    jd = j - 4 * c
                    vco = jd * 128 if jd > 0 else 0
                    vcos.append(vco)
                    kt = kt_t[hp][j // 4]
                    kc = (j % 4) * 128
                    ss = ps_s.tile([128, 1024], f32, name="s", tag="s")
                    nc.tensor.matmul(ss[:, vco:512],
                                     kt[0:64, kc:kc + 128],
                                     qt_c[hp][0:64, vco:512],
                                     start=True, stop=True)
                    nc.tensor.matmul(ss[:, 512 + vco:1024],
                                     kt[64:128, kc:kc + 128],
                                     qt_c[hp][64:128, vco:512],
                                     start=True, stop=True)
                    emit_exp(ss, et, i, vco)
                # i-major: both heads' AV for key tile i=0 are queued before
                # the exp(i=1)-dependent matmuls, so the PE FIFO never holds
                # a blocked matmul ahead of an unblocked one.
                for i in range(2):
                    vco = vcos[i]
                    for h in range(2):
                        gh = 2 * hp + h
                        co = h * 512
                        vsl = v_t[jp][:, i, gh * 66:gh * 66 + 65]
                        esl = et[:, i, co + vco:co + 512]
                        nc.tensor.matmul(av[h][0:65, vco:512], vsl, esl,
                                         start=(first and i == 0),
                                         stop=(last and i == 1))
                if jp % stride == stride - 1 and fillers:
                    fillers.popleft()()
            # keep the PE fed while the DVE sums-copy / recip chain runs:
            # pop fillers BEFORE the rc_rep matmuls (PE executes in emission
            # order, so anything emitted after rc_rep would stall behind it)
            if fillers:
                fillers.popleft()()
            # ---- normalization: row 64 of av[h] is the denominator ----
            # both heads' sums stage at partitions 0/32 of one tile; one
            # block matmul replicates h0 to rows 0:64 and h1 to rows
            # 64:128, so the reciprocal is a single [128, 512] pass.
            unit_ctr[0] += 1
            sums = sums_t[unit_ctr[0] % 2]
            # h0 copy on DVE, h1 copy + reciprocal on ACT: the chain hops
            # engines so each step starts on its semaphore instead of
            # queueing, and the av banks free up sooner for the next unit.
            nc.vector.tensor_copy(sums[0:1, :], av[0][64:65, :])
            nc.scalar.activation(sums[32:33, :], av[1][64:65, :],
                                 mybir.ActivationFunctionType.Copy)
            rr = ps_mm.tile([128, 512], f32, name="rr", tag="mm")
            nc.tensor.matmul(rr[:], ones2_t[:], sums[:],
                             start=True, stop=True)
            rcf = rcf_pool.tile([128, 512], f32, name="rcf", tag="rcf")
            nc.vector.reciprocal_approx_fast(rcf[:], rr[:])
            for h in range(2):
                nc.vector.tensor_mul(at_t[c][h * 64:(h + 1) * 64, hp, :],
                                     av[h][0:64, :],
                                     rcf[h * 64:(h + 1) * 64, :])

        phases = [
            ([(0, 0), (0, 1), (0, 2), (0, 3)], qkv_units(1)),
            ([(1, 0), (1, 1), (1, 2), (1, 3)], qkv_units(2)),
            ([(2, 0), (2, 1), (2, 2), (2, 3)], qkv_units(3)),
            ([(3, 0), (3, 1), (3, 2), (3, 3)],
             proj_units(0) + proj_units(1) + proj_units(2)),
        ]
        for units, filler_list in phases:
            fillers = deque(filler_list)
            total_jp = sum(2 * c + 2 for c, hp in units)
            stride = max(1, -(-total_jp // max(1, len(filler_list))))
            for (c, hp) in units:
                attn_unit(c, hp, fillers, stride)
            while fillers:
                fillers.popleft()()

        # tail: last chunk's proj through the now-idle ps_s pool. Each
        # 512-wide half evacuates and stores as soon as its 4-matmul chain
        # stops, so the final evac/store drains only half a tile.
        c = NCH - 1
        for tl in range(4):
            t = 4 * c + tl
            ps = ps_s.tile([128, 1024], f32, name="s", tag="s")
            ost = ost_pool.tile([128, 1024], f32, name="ost2", tag="ost2")
            # k0..k2 of BOTH n2 chains first: at[:, 3, :] comes from the
            # last attention unit's normalize, so deferring the k=3
            # matmuls hides that wait behind six ready matmuls.
            for k in range(3):
                for n2 in range(2):
                    nc.tensor.matmul(
                        ps[:, n2 * 512:(n2 + 1) * 512],
                        at_t[c][:, k, tl * 128:(tl + 1) * 128],
                        wp_t[:, k, n2 * 512:(n2 + 1) * 512],
                        start=(k == 0), stop=False)
            for n2 in range(2):
                nc.tensor.matmul(
                    ps[:, n2 * 512:(n2 + 1) * 512],
                    at_t[c][:, 3, tl * 128:(tl + 1) * 128],
                    wp_t[:, 3, n2 * 512:(n2 + 1) * 512],
                    start=False, stop=True)
                nc.vector.tensor_copy(ost[:, n2 * 512:(n2 + 1) * 512],
                                      ps[:, n2 * 512:(n2 + 1) * 512])
                q = nc.sync if n2 == 0 else nc.scalar
                q.dma_start(out[t * 128:(t + 1) * 128,
                                n2 * 512:(n2 + 1) * 512],
                            ost[:, n2 * 512:(n2 + 1) * 512])

    nc.compile()
    return nc


def _get_nc():
    if "nc" not in _CACHE:
        _CACHE["nc"] = _build()
    return _CACHE["nc"]


def _ones2_host():
    o = np.zeros((64, 128), dtype=ml_dtypes.bfloat16)
    o[0, 0:64] = 1.0
    o[32, 64:128] = 1.0
    return o


def _in_maps(x, w_qkv, w_proj):
    f8np = ml_dtypes.float8_e4m3
    bf16np = ml_dtypes.bfloat16
    x_np = f8np if QKV_FP8 else bf16np
    at_np = f8np if PROJ_FP8 else bf16np
    wsc = WSCALE if QKV_FP8 else 1.0
    wsp = WSCALE if PROJ_FP8 else 1.0

    wr = w_qkv.reshape(C, 3, H, HD)
    wpr = w_proj.reshape(H, HD, C)
    maps = []
    for core in range(NCORES):
        b, g = core // 2, core % 2
        hs = slice(g * HPC, (g + 1) * HPC)
        maps.append({
            "xT": np.ascontiguousarray(x[b].T).astype(x_np),
            "wq": (wr[:, 0, hs, :].reshape(C, F) * wsc).astype(x_np),
            "wk": (wr[:, 1, hs, :].reshape(C, F) * wsc).astype(x_np),
            "wv": (wr[:, 2, hs, :].reshape(C, F) * wsc).astype(x_np),
            "wp": (wpr[hs].reshape(F, C) * wsp).astype(at_np),
            "ones2": _ones2_host(),
        })
    return maps


def kernel(x, w_qkv, w_proj, b_proj, _trace=False):
    from concourse.bass_utils import run_bass_kernel_spmd

    x = np.asarray(x, dtype=np.float32)
    w_qkv = np.asarray(w_qkv, dtype=np.float32)
    w_proj = np.asarray(w_proj, dtype=np.float32)
    b_proj = np.asarray(b_proj, dtype=np.float32)

    nc = _get_nc()
    in_maps = _in_maps(x, w_qkv, w_proj)
    try:
        res = run_bass_kernel_spmd(nc, in_maps, list(range(NCORES)),
                                   trace=_trace)
    except Exception:
        try:
            import ctypes
            import jax
            lib = ctypes.CDLL("/opt/axon/libaxon_pjrt.so")
            jax.devices()
            lib.axon_reset.restype = ctypes.c_int64
            lib.axon_reset()
        except Exception:
            pass
        res = run_bass_kernel_spmd(nc, in_maps, list(range(NCORES)),
                                   trace=_trace)
    unscale = 1.0
    if QKV_FP8:
        unscale *= WSCALE        # v path carries one WSCALE into at
    if PROJ_FP8:
        unscale *= WSCALE        # wp carries another
    out = np.empty((B, N, C), dtype=np.float32)
    for b in range(B):
        out[b] = res.results[2 * b]["out"] + res.results[2 * b + 1]["out"]
    if unscale != 1.0:
        out *= 1.0 / unscale
    out += b_proj.reshape(1, 1, C)
    if _trace:
        return out, res
    return out



# revision 58
# speedup vs baseline: 1.0113x; 1.0049x over previous
"""Block-causal attention (B=4, N=2048, C=1024, H=16, block=128) on 8 TRN2
NeuronCores — bf16 edition.

Sharding: core = 2*b + g (b in 0..3 batches, g in 0..1 head-groups of 8
heads). Feature-major q/k, token-major v, out^T attention accumulation,
per-core partial projection; host sums the two half-feature partials.

- All matmuls bf16 (fp8 e4m3 fails the 2e-2 gate: any fp8 stage measures
  4e-2..1e-1 max-rel-err in numpy simulation — flags kept for reference).
- QK pairs use 64-row contraction at base partitions 0/64, so the two
  heads' score matmuls run concurrently in separate PE row groups.
- attn@v uses a ones-augmented v (M=65): row 64 of each AV accumulator IS
  the softmax denominator — no separate denominator pass.
- Normalization: both heads' denominator rows stage at partitions 0/32 of
  a zeroed [64, 512] tile; ONE block matmul (host [64, 128] replicator)
  replicates h0 to rows 0:64 / h1 to 64:128, one [128, 512] reciprocal,
  two muls into the bf16 `at` tile that feeds proj. rr lives in the ps_mm
  pool so this chain never blocks the next unit's QK score matmuls.
- exp runs on ACT (exp(S)/32 with per-partition bias); every 4th tile is
  offloaded to DVE via a Schraudolph bitcast exp (i16 saturating convert).
- x chunks and wk load via 4 sub-DMAs each (fine-grained sems for early
  start); wq/wv/wp load in one dma_start each.
- Tail: last chunk's proj runs through the idle ps_s pool with [128, 1024]
  merged evacuations and stores.
"""

import numpy as np
import ml_dtypes
from collections import deque
from contextlib import ExitStack

B, N, C, H, HD = 4, 2048, 1024, 16, 64
HPC = 8               # heads per core
F = HPC * HD          # 512 features per core
NCORES = 8
SCALE = float(HD) ** -0.5
NT = N // 128         # 16 token tiles
NCH = 4               # token chunks of 512

QKV_FP8 = False
ET_FP8 = False
PROJ_FP8 = False
WSCALE = 32.0         # host pre-scale on wq/wk/wv/wp before fp8 quant
DVE_EXP_MOD = 4       # every DVE_EXP_MOD-th exp tile runs on DVE (0 = off)

LNK = 3.4657359027997265  # ln(32): exp(s)/32 keeps max logit 8.06 under fp8e4 max 240
LOG2E = 1.4426950408889634

_CACHE = {}


def _build():
    import concourse.mybir as mybir
    import concourse.tile as tile
    from concourse import bacc

    f32 = mybir.dt.float32
    f32r = mybir.dt.float32r
    bf16 = mybir.dt.bfloat16
    f8 = mybir.dt.float8e4
    u8 = mybir.dt.uint8
    i16 = mybir.dt.int16
    Exp = mybir.ActivationFunctionType.Exp
    DR = mybir.MatmulPerfMode.DoubleRow
    MULT = mybir.AluOpType.mult
    ADD = mybir.AluOpType.add

    qk_ws = WSCALE * WSCALE if QKV_FP8 else 1.0
    scale_eff = SCALE / qk_ws
    et_dt = f8 if ET_FP8 else bf16
    at_dt = f8 if PROJ_FP8 else bf16
    K8 = scale_eff * LOG2E * 8.0
    B8 = (7.0 - 5.0) * 8.0 - 0.5
    K16 = scale_eff * LOG2E * 128.0
    B16 = 127.0 * 128.0 - 0.0579 * 128.0 - 5.0 * 128.0  # incl. exp/32

    nc = bacc.Bacc("TRN2", target_bir_lowering=False, debug=False,
                   num_devices=NCORES)

    x_dt = f8 if QKV_FP8 else bf16
    xT = nc.dram_tensor("xT", [C, N], x_dt, kind="ExternalInput")
    wq = nc.dram_tensor("wq", [C, F], x_dt, kind="ExternalInput")
    wk = nc.dram_tensor("wk", [C, F], x_dt, kind="ExternalInput")
    wv = nc.dram_tensor("wv", [C, F], x_dt, kind="ExternalInput")
    wp = nc.dram_tensor("wp", [F, C], at_dt, kind="ExternalInput")
    ones2d = nc.dram_tensor("ones2", [64, 128], bf16, kind="ExternalInput")
    out = nc.dram_tensor("out", [N, C], f32, kind="ExternalOutput")

    with tile.TileContext(nc) as tc, ExitStack() as ctx:
        persist = ctx.enter_context(tc.tile_pool(name="persist", bufs=1))
        xt_pool = ctx.enter_context(tc.tile_pool(name="xt", bufs=2))
        qt_pool = ctx.enter_context(tc.tile_pool(name="qt", bufs=2))
        et_pool = ctx.enter_context(tc.tile_pool(name="et", bufs=3))
        sm_pool = ctx.enter_context(tc.tile_pool(name="sums", bufs=2))
        rcf_pool = ctx.enter_context(tc.tile_pool(name="rcf", bufs=2))
        ost_pool = ctx.enter_context(tc.tile_pool(name="ost", bufs=2))
        ps_mm = ctx.enter_context(tc.tile_pool(name="ps_mm", bufs=2, space="PSUM"))
        ps_s = ctx.enter_context(tc.tile_pool(name="ps_s", bufs=2, space="PSUM"))
        ps_av = ctx.enter_context(tc.tile_pool(name="ps_av", bufs=1, space="PSUM"))

        # ---- persistent weights ----
        if QKV_FP8:
            wq_t = [persist.tile([128, 2, F], f8, name=f"wq{p}", tag=f"wq{p}")
                    for p in range(4)]
            wk_t = [persist.tile([128, 2, F], f8, name=f"wk{p}", tag=f"wk{p}")
                    for p in range(4)]
            wv_t = [persist.tile([128, 2, F], f8, name=f"wv{p}", tag=f"wv{p}")
                    for p in range(4)]
        else:
            # fused [128, 8, F] so each weight tensor loads in ONE dma_start
            wq_t = persist.tile([128, 8, F], bf16, name="wqf", tag="wqf")
            wk_t = persist.tile([128, 8, F], bf16, name="wkf", tag="wkf")
            wv_t = persist.tile([128, 8, F], bf16, name="wvf", tag="wvf")
        if PROJ_FP8:
            wp_t = [persist.tile([128, 2, C], f8, name=f"wp{p}", tag=f"wp{p}")
                    for p in range(2)]
        else:
            wp_t = persist.tile([128, 4, C], bf16, name="wpf", tag="wpf")
        # block replicator: row 0 is 1 on cols 0:64, row 32 is 1 on cols
        # 64:128, 0 elsewhere — one K=64 matmul replicates both heads'
        # denominators at once. The head sums stage at partitions 0 and 32
        # (engine writes must start at 32-aligned partitions); the other
        # rows stay 0 from the startup memset.
        ones2_t = persist.tile([64, 128], bf16, name="ones2", tag="ones2")
        # garbage operand for the HAM warm-up burst (memset once, never
        # DMA'd, so the warm-up matmuls depend on nothing but the memset)
        warm_t = persist.tile([128, 512], bf16, name="warm", tag="warm")
        sums_t = [persist.tile([64, 512], bf16, name=f"sums{i}",
                               tag=f"sums{i}") for i in range(2)]
        bias_t = persist.tile([128, 1], f32, name="bias", tag="bias")

        # persistent k^T (bf16) and ones-augmented token-major v (pairs)
        kt_t = [[persist.tile([128, 512], bf16, name=f"kT{hp}_{jc}",
                              tag=f"kT{hp}_{jc}")
                 for jc in range(NCH)] for hp in range(4)]
        # 66-wide per-head v slices: 64 feature cols + ones col + pad col
        # (even head offsets/strides; dual-fp8 ldweights requires them)
        v_t = [persist.tile([128, 2, 8 * 66], et_dt, name=f"v{jp}",
                            tag=f"v{jp}") for jp in range(NT // 2)]
        at_t = [persist.tile([128, 4, 512], at_dt, name=f"at{c}", tag=f"at{c}")
                for c in range(NCH)]

        def load_weights():
            nc.gpsimd.dma_start(ones2_t[:], ones2d[:, :])
            nc.vector.memset(sums_t[0][:], 0.0)
            nc.vector.memset(sums_t[1][:], 0.0)
            nc.vector.memset(bias_t[:], -LNK)
            for jp in range(NT // 2):
                # col 64 of each head's 65-wide v slice must be 1.0; the v
                # copies later only overwrite cols 0:64, so setting cols
                # 64:66 (ones + pad) up-front leaves them in place forever.
                # Vector queue: keeps gpsimd free to start weight DMAs.
                ones_cols = v_t[jp][:].rearrange(
                    "p i (h e) -> p i h e", e=66)[:, :, :, 64:66]
                nc.vector.memset(ones_cols, 1.0)
            # wk gates the very first matmuls: split 4-way so the k=0 pair
            # lands early. wv/wq/wp are needed later; one dma_start each.
            wkr = wk[:, :].rearrange("(k p) f -> p k f", p=128)
            wvr = wv[:, :].rearrange("(k p) f -> p k f", p=128)
            wqr = wq[:, :].rearrange("(k p) f -> p k f", p=128)
            for q in range(4):
                nc.scalar.dma_start(wk_t[:, 2 * q:2 * q + 2, :],
                                    wkr[:, 2 * q:2 * q + 2, :])
            for q in range(4):
                nc.gpsimd.dma_start(wv_t[:, 2 * q:2 * q + 2, :],
                                    wvr[:, 2 * q:2 * q + 2, :])
            for q in range(2):
                nc.sync.dma_start(wq_t[:, 4 * q:4 * q + 4, :],
                                  wqr[:, 4 * q:4 * q + 4, :])
            # wp is needed only at the first proj fillers; queue it on the
            # scalar ring BEHIND wk so it doesn't steal early DMA bandwidth
            # from the xt/wk/wv transfers that gate the first matmuls.
            nc.scalar.dma_start(
                wp_t[:], wp[:, :].rearrange("(k p) c -> p k c", p=128))

        qt_state = {c: [] for c in range(NCH)}
        exp_ctr = [0]
        mm_ctr = [0]
        unit_ctr = [0]

        def mm_tile(c):
            # chunk 0 runs before any attention, so the ss banks are idle;
            # alternating into them deepens the qkv pipeline 2 -> 4 buffers.
            mm_ctr[0] += 1
            if c == 0 and mm_ctr[0] % 2 == 0:
                return ps_s.tile([128, 1024], f32, name="s", tag="s")[:, 0:512]
            return ps_mm.tile([128, 512], f32, name="mm", tag="mm")[:]

        def qkv_units(c):
            c0 = c * 512
            xt_c = []

            def load():
                # 4 dma_starts per chunk: fine-grained sems let the k=0
                # matmuls start while later k-slices are still in flight.
                xt = xt_pool.tile([128, 8, 512], bf16, name="xt", tag="xt")
                xr = xT[:, c0:c0 + 512].rearrange("(k p) n -> p k n", p=128)
                for q in range(4):
                    nc.sync.dma_start(xt[:, 2 * q:2 * q + 2, :],
                                      xr[:, 2 * q:2 * q + 2, :])
                xt_c.append(xt)

            def qk_mm(ps, w_t, hp):
                for k in range(8):
                    nc.tensor.matmul(ps[:],
                                     w_t[:, k, hp * 128:(hp + 1) * 128],
                                     xt_c[0][:, k, :],
                                     start=(k == 0), stop=(k == 7))

            def q_group(hp):
                def emit():
                    ps = mm_tile(c)
                    qk_mm(ps, wq_t, hp)
                    qt = qt_pool.tile([128, 512], bf16, name=f"qT{hp}",
                                      tag=f"qT{hp}")
                    nc.vector.tensor_copy(qt[:], ps[:])
                    qt_state[c].append(qt)
                return emit

            def k_group(hp):
                def emit():
                    ps = mm_tile(c)
                    qk_mm(ps, wk_t, hp)
                    nc.vector.tensor_copy(kt_t[hp][c][:], ps[:])
                return emit

            def v_group(tl):
                def emit():
                    t = 4 * c + tl
                    ps = mm_tile(c)
                    for k in range(8):
                        nc.tensor.matmul(ps[:],
                                         xt_c[0][:, k, tl * 128:(tl + 1) * 128],
                                         wv_t[:, k, :],
                                         start=(k == 0), stop=(k == 7))
                    src = ps[:].rearrange("p (h e) -> p h e", e=64)
                    dst = v_t[t // 2][:, t % 2, :].rearrange(
                        "p (h e) -> p h e", e=66)[:, :, 0:64]
                    nc.vector.tensor_copy(dst, src)
                return emit

            units = [load]
            for hp in range(4):
                units.append(k_group(hp))
            for tl in range(4):
                units.append(v_group(tl))
            for hp in range(4):
                units.append(q_group(hp))
            return units

        def proj_units(c):
            units = []
            for tl in range(4):
                for n2 in range(2):
                    def emit(tl=tl, n2=n2):
                        t = 4 * c + tl
                        ps = ps_mm.tile([128, 512], f32, name="mm", tag="mm")
                        for k in range(4):
                            nc.tensor.matmul(
                                ps[:],
                                at_t[c][:, k, tl * 128:(tl + 1) * 128],
                                wp_t[:, k, n2 * 512:(n2 + 1) * 512],
                                start=(k == 0), stop=(k == 3))
                        ost = ost_pool.tile([128, 512], f32, name="ost",
                                            tag="ost")
                        nc.vector.tensor_copy(ost[:], ps[:])
                        nc.sync.dma_start(
                            out[t * 128:(t + 1) * 128,
                                n2 * 512:(n2 + 1) * 512],
                            ost[:])
                    units.append(emit)
            return units

        units0 = qkv_units(0)
        units0[0]()
        # HAM warm-up: ~10us of garbage matmuls (8 cold + 32 warm) bridge
        # the whole first-chunk DMA wait, so the clock gate is at 8/8 and
        # stays there when the real stream starts at ~12us. All of this
        # hides inside the unavoidable DMA window.
        nc.vector.memset(warm_t[:], 0.125)
        wps = ps_mm.tile([128, 512], f32, name="mm", tag="mm")
        for _ in range(40):
            nc.tensor.matmul(wps[:], warm_t[0:128, 0:128], warm_t[:, 0:512],
                             start=True, stop=True)
        load_weights()
        for u in units0[1:]:
            u()

        def emit_exp(ss, et, i, vco):
            src = ss[:].rearrange("p (h q) -> p h q", h=2)[:, :, vco:512]
            dst = et[:, i, :].rearrange("p (h q) -> p h q", h=2)[:, :, vco:512]
            exp_ctr[0] += 1
            on_dve = DVE_EXP_MOD and (exp_ctr[0] % DVE_EXP_MOD == 0)
            if on_dve:
                if ET_FP8:
                    nc.vector.tensor_scalar(dst.bitcast(u8), src, K8, B8,
                                            MULT, ADD)
                else:
                    nc.vector.tensor_scalar(dst.bitcast(i16), src, K16, B16,
                                            MULT, ADD)
            else:
                nc.scalar.activation(dst, src, Exp, bias=bias_t[:],
                                     scale=scale_eff)

        def attn_unit(c, hp, fillers, stride=2):
            njp = 2 * c + 2
            qt_c = qt_state[c]
            av = [ps_av.tile([128, 512], f32, name=f"av{h}", tag=f"av{h}")
                  for h in range(2)]
            for jp in range(njp):
                first, last = (jp == 0), (jp == njp - 1)
                et = et_pool.tile([128, 2, 1024], et_dt, name="e", tag="e")
                vcos = []
                for i in range(2):
                    j = 2 * jp + i
                    jd = j - 4 * c
                    vco = jd * 128 if jd > 0 else 0
                    vcos.append(vco)
                    kt = kt_t[hp][j // 4]
                    kc = (j % 4) * 128
                    ss = ps_s.tile([128, 1024], f32, name="s", tag="s")
                    nc.tensor.matmul(ss[:, vco:512],
                                     kt[0:64, kc:kc + 128],
                                     qt_c[hp][0:64, vco:512],
                                     start=True, stop=True)
                    nc.tensor.matmul(ss[:, 512 + vco:1024],
                                     kt[64:128, kc:kc + 128],
                                     qt_c[hp][64:128, vco:512],
                                     start=True, stop=True)
                    emit_exp(ss, et, i, vco)
                # i-major: both heads' AV for key tile i=0 are queued before
                # the exp(i=1)-dependent matmuls, so the PE FIFO never holds
                # a blocked matmul ahead of an unblocked one.
                for i in range(2):
                    vco = vcos[i]
                    for h in range(2):
                        gh = 2 * hp + h
                        co = h * 512
                        vsl = v_t[jp][:, i, gh * 66:gh * 66 + 65]
                        esl = et[:, i, co + vco:co + 512]
                        nc.tensor.matmul(av[h][0:65, vco:512], vsl, esl,
                                         start=(first and i == 0),
                                         stop=(last and i == 1))
                if jp % stride == stride - 1 and fillers:
                    fillers.popleft()()
            # keep the PE fed while the DVE sums-copy / recip chain runs:
            # pop fillers BEFORE the rc_rep matmuls (PE executes in emission
            # order, so anything emitted after rc_rep would stall behind it)
            if fillers:
                fillers.popleft()()
            # ---- normalization: row 64 of av[h] is the denominator ----
            # both heads' sums stage at partitions 0/32 of one tile; one
            # block matmul replicates h0 to rows 0:64 and h1 to rows
            # 64:128, so the reciprocal is a single [128, 512] pass.
            unit_ctr[0] += 1
            sums = sums_t[unit_ctr[0] % 2]
            # h0 copy on DVE, h1 copy + reciprocal on ACT: the chain hops
            # engines so each step starts on its semaphore instead of
            # queueing, and the av banks free up sooner for the next unit.
            nc.vector.tensor_copy(sums[0:1, :], av[0][64:65, :])
            nc.scalar.activation(sums[32:33, :], av[1][64:65, :],
                                 mybir.ActivationFunctionType.Copy)
            rr = ps_mm.tile([128, 512], f32, name="rr", tag="mm")
            nc.tensor.matmul(rr[:], ones2_t[:], sums[:],
                             start=True, stop=True)
            rcf = rcf_pool.tile([128, 512], f32, name="rcf", tag="rcf")
            nc.vector.reciprocal_approx_fast(rcf[:], rr[:])
            for h in range(2):
                nc.vector.tensor_mul(at_t[c][h * 64:(h + 1) * 64, hp, :],
                                     av[h][0:64, :],
                                     rcf[h * 64:(h + 1) * 64, :])

        phases = [
            ([(0, 0), (0, 1), (0, 2), (0, 3)], qkv_units(1)),
            ([(1, 0), (1, 1), (1, 2), (1, 3)], qkv_units(2)),
            ([(2, 0), (2, 1), (2, 2), (2, 3)], qkv_units(3)),
            ([(3, 0), (3, 1), (3, 2), (3, 3)],
             proj_units(0) + proj_units(1) + proj_units(2)),
        ]
        for units, filler_list in phases:
            fillers = deque(filler_list)
            total_jp = sum(2 * c + 2 for c, hp in units)
            stride = max(1, -(-total_jp // max(1, len(filler_list))))
            for (c, hp) in units:
                attn_unit(c, hp, fillers, stride)
            while fillers:
                fillers.popleft()()

        # tail: last chunk's proj through the now-idle ps_s pool. Each
        # 512-wide half evacuates and stores as soon as its 4-matmul chain
        # stops, so the final evac/store drains only half a tile.
        c = NCH - 1
        for tl in range(4):
            t = 4 * c + tl
            ps = ps_s.tile([128, 1024], f32, name="s", tag="s")
            ost = ost_pool.tile([128, 1024], f32, name="ost2", tag="ost2")
            for n2 in range(2):
                for k in range(4):
                    nc.tensor.matmul(
                        ps[:, n2 * 512:(n2 + 1) * 512],
                        at_t[c][:, k, tl * 128:(tl + 1) * 128],
                        wp_t[:, k, n2 * 512:(n2 + 1) * 512],
                        start=(k == 0), stop=(k == 3))
                nc.vector.tensor_copy(ost[:, n2 * 512:(n2 + 1) * 512],
                                      ps[:, n2 * 512:(n2 + 1) * 512])
                q = nc.sync if n2 == 0 else nc.scalar
                q.dma_start(out[t * 128:(t + 1) * 128,
                                n2 * 512:(n2 + 1) * 512],
                            ost[:, n2 * 512:(n2 + 1) * 512])

    nc.compile()
    return nc


def _get_nc():
    if "nc" not in _CACHE:
        _CACHE["nc"] = _build()
    return _CACHE["nc"]


def _ones2_host():
    o = np.zeros((64, 128), dtype=ml_dtypes.bfloat16)
    o[0, 0:64] = 1.0
    o[32, 64:128] = 1.0
    return o


def _in_maps(x, w_qkv, w_proj):
    f8np = ml_dtypes.float8_e4m3
    bf16np = ml_dtypes.bfloat16
    x_np = f8np if QKV_FP8 else bf16np
    at_np = f8np if PROJ_FP8 else bf16np
    wsc = WSCALE if QKV_FP8 else 1.0
    wsp = WSCALE if PROJ_FP8 else 1.0

    wr = w_qkv.reshape(C, 3, H, HD)
    wpr = w_proj.reshape(H, HD, C)
    maps = []
    for core in range(NCORES):
        b, g = core // 2, core % 2
        hs = slice(g * HPC, (g + 1) * HPC)
        maps.append({
            "xT": np.ascontiguousarray(x[b].T).astype(x_np),
            "wq": (wr[:, 0, hs, :].reshape(C, F) * wsc).astype(x_np),
            "wk": (wr[:, 1, hs, :].reshape(C, F) * wsc).astype(x_np),
            "wv": (wr[:, 2, hs, :].reshape(C, F) * wsc).astype(x_np),
            "wp": (wpr[hs].reshape(F, C) * wsp).astype(at_np),
            "ones2": _ones2_host(),
        })
    return maps


def kernel(x, w_qkv, w_proj, b_proj, _trace=False):
    from concourse.bass_utils import run_bass_kernel_spmd

    x = np.asarray(x, dtype=np.float32)
    w_qkv = np.asarray(w_qkv, dtype=np.float32)
    w_proj = np.asarray(w_proj, dtype=np.float32)
    b_proj = np.asarray(b_proj, dtype=np.float32)

    nc = _get_nc()
    in_maps = _in_maps(x, w_qkv, w_proj)
    try:
        res = run_bass_kernel_spmd(nc, in_maps, list(range(NCORES)),
                                   trace=_trace)
    except Exception:
        try:
            import ctypes
            import jax
            lib = ctypes.CDLL("/opt/axon/libaxon_pjrt.so")
            jax.devices()
            lib.axon_reset.restype = ctypes.c_int64
            lib.axon_reset()
        except Exception:
            pass
        res = run_bass_kernel_spmd(nc, in_maps, list(range(NCORES)),
                                   trace=_trace)
    unscale = 1.0
    if QKV_FP8:
        unscale *= WSCALE        # v path carries one WSCALE into at
    if PROJ_FP8:
        unscale *= WSCALE        # wp carries another
    out = np.empty((B, N, C), dtype=np.float32)
    for b in range(B):
        out[b] = res.results[2 * b]["out"] + res.results[2 * b + 1]["out"]
    if unscale != 1.0:
        out *= 1.0 / unscale
    out += b_proj.reshape(1, 1, C)
    if _trace:
        return out, res
    return out

